# revision 18
# baseline (speedup 1.0000x reference)
"""Trainium2 Bass kernel for nn_CPVTV2 block (B=16,N=1024,C=512), data-parallel on 8 cores.

Self-contained: imports only the installed /opt/trn_rl_repo environment.
"""
import sys

if "/opt/trn_rl_repo" not in sys.path:
    sys.path.insert(0, "/opt/trn_rl_repo")

from contextlib import ExitStack

import ml_dtypes
import numpy as np

import concourse.bass as bass
import concourse.tile as tile
from concourse import bacc, mybir
from concourse.bass_utils import run_bass_kernel_spmd
from concourse.masks import make_identity

F32 = mybir.dt.float32
F32R = mybir.dt.float32r
BF16 = mybir.dt.bfloat16
AF = mybir.ActivationFunctionType
OP = mybir.AluOpType

# problem constants (per core after B-sharding over 8 cores)
BL = 2            # batches per core
N = 1024          # tokens per batch
C = 512           # channels
HID = 2048
CA = 128
T = BL * N        # 2048 tokens per core
P = 128
CCH = C // P      # 4 channel chunks
HCH = HID // P    # 16 hidden chunks
NT = 512          # token-tile width (free dim) for feature-major matmuls
G_PER_B = N // P  # 8 groups of 128 tokens per batch
NTILES_B = N // NT  # 2 token-tiles per batch
EPS = 1e-5
INV_C1 = 1.0 / (C - 1)


def _build(c1, c2, b2, gates):
    """Build + compile the Bass program. gates: dict of bools for optional bias paths."""
    nc = bacc.Bacc("TRN2", target_bir_lowering=False, debug=False, num_devices=8)

    def din(name, shape, dt):
        return nc.dram_tensor(name, list(shape), dt, kind="ExternalInput").ap()

    x_d = din("x", [T, C], F32)
    out_d = nc.dram_tensor("out", [T, C], F32, kind="ExternalOutput").ap()
    wqk_d = din("wqkT", [C, 2 * C], F32R)
    wv_d = din("wvT", [C, C], F32R)
    pl_d = din("plT", [C, C], F32R)
    fc1_d = din("fc1T", [C, HID], BF16)
    fc2_d = din("fc2T", [HID, C], BF16)
    fc1b_d = din("fc1b", [HID], F32)
    ca1_d = din("ca1T", [C, CA], F32R)
    ca2_d = din("ca2T", [CA, C], F32R)
    cab1_d = din("cab1", [CA, 1], F32)
    cab2_d = din("cab2", [P, CCH], F32)
    g1r_d = din("g1row", [1, C], F32)
    b1r_d = din("b1row", [1, C], F32)
    vb_d = din("vbcol", [P, CCH], F32)
    w1d_d = din("w1d", [BL, 3, 4 * BL], F32R)     # conv1 lhsT per shift: [2, 8]
    w2d_d = din("w2d", [4 * BL, 3, BL], F32R)     # conv2 lhsT per shift: [8, 2]
    p8_d = din("p8", [4 * BL, 4 * BL], F32R)
    gng_d = din("gng", [4 * BL, 1], F32)
    gnb_d = din("gnb", [4 * BL, 1], F32)
    ones_d = din("onesc", [P, 2], F32R)
    zeros_d = din("zeros8", [4 * BL, 1], F32R)
    if gates["qkb"]:
        qb_d = din("qbrow", [1, 2 * C], F32)
    if gates["plb"]:
        plb_d = din("plbrow", [1, C], F32)
    if gates["fc2b"]:
        fc2b_d = din("fc2brow", [1, C], F32)
    if gates["ln2b"]:
        ln2b_d = din("ln2brow", [1, C], F32)

    with tile.TileContext(nc) as tc, ExitStack() as ctx:
        sg = ctx.enter_context(tc.tile_pool(name="singles", bufs=1))
        wk = ctx.enter_context(tc.tile_pool(name="work", bufs=2))
        wk3 = ctx.enter_context(tc.tile_pool(name="work3", bufs=3))
        wk4 = ctx.enter_context(tc.tile_pool(name="work4", bufs=5))
        ps_tr = ctx.enter_context(tc.tile_pool(name="ps_tr", bufs=2, space="PSUM"))
        ps_qk = ctx.enter_context(tc.tile_pool(name="ps_qk", bufs=2, space="PSUM"))
        ps_big = ctx.enter_context(tc.tile_pool(name="ps_big", bufs=3, space="PSUM"))
        ps_sm = ctx.enter_context(tc.tile_pool(name="ps_sm", bufs=1, space="PSUM"))

        dma = nc.sync.dma_start

        # ---------- resident weights/constants ----------
        wqk_sb = sg.tile([P, CCH, 2 * C], F32R, tag="wqk")
        dma(wqk_sb[:], wqk_d.rearrange("(cc p) f -> p cc f", p=P))
        wv_sb = sg.tile([P, CCH, C], F32R, tag="wv")
        dma(wv_sb[:], wv_d.rearrange("(cc p) f -> p cc f", p=P))
        pl_sb = sg.tile([P, CCH, C], F32R, tag="pl")
        dma(pl_sb[:], pl_d.rearrange("(cc p) f -> p cc f", p=P))
        fc1_sb = sg.tile([P, CCH, HID], BF16, tag="fc1")
        dma(fc1_sb[:], fc1_d.rearrange("(cc p) f -> p cc f", p=P))
        fc2_sb = sg.tile([P, HCH, C], BF16, tag="fc2")
        dma(fc2_sb[:], fc2_d.rearrange("(hc p) f -> p hc f", p=P))
        fc1b_sb = sg.tile([P, HCH], F32, tag="fc1b")
        dma(fc1b_sb[:], fc1b_d.rearrange("(hc p) -> p hc", p=P))
        ca1_sb = sg.tile([P, CCH, CA], F32R, tag="ca1")
        dma(ca1_sb[:], ca1_d.rearrange("(cc p) f -> p cc f", p=P))
        ca2_sb = sg.tile([CA, CCH, P], F32R, tag="ca2")
        dma(ca2_sb[:], ca2_d.rearrange("j (cc c) -> j cc c", cc=CCH))
        cab1_sb = sg.tile([CA, 1], F32, tag="cab1")
        dma(cab1_sb[:], cab1_d)
        cab2_sb = sg.tile([P, CCH], F32, tag="cab2")
        dma(cab2_sb[:], cab2_d)
        g1r_sb = sg.tile([1, C], F32, tag="g1r")
        dma(g1r_sb[:], g1r_d)
        b1r_sb = sg.tile([1, C], F32, tag="b1r")
        dma(b1r_sb[:], b1r_d)
        vb_sb = sg.tile([P, CCH], F32, tag="vb")
        dma(vb_sb[:], vb_d)
        w1d_sb = sg.tile([BL, 3, 4 * BL], F32R, tag="w1d")
        dma(w1d_sb[:], w1d_d)
        w2d_sb = sg.tile([4 * BL, 3, BL], F32R, tag="w2d")
        dma(w2d_sb[:], w2d_d)
        p8_sb = sg.tile([4 * BL, 4 * BL], F32R, tag="p8")
        dma(p8_sb[:], p8_d)
        gng_sb = sg.tile([4 * BL, 1], F32, tag="gng")
        dma(gng_sb[:], gng_d)
        gnb_sb = sg.tile([4 * BL, 1], F32, tag="gnb")
        dma(gnb_sb[:], gnb_d)
        ones_sb = sg.tile([P, 2], F32R, tag="ones")
        dma(ones_sb[:], ones_d)
        if gates["qkb"]:
            qb_sb = sg.tile([P, 2 * C], F32, tag="qb")
            nc.gpsimd.dma_start(out=qb_sb[:], in_=bass.AP(
                tensor=qb_d.tensor, offset=0, ap=[[0, P], [1, 2 * C]]))
        if gates["plb"]:
            plb_sb = sg.tile([P, C], F32, tag="plb")
            nc.gpsimd.dma_start(out=plb_sb[:], in_=bass.AP(
                tensor=plb_d.tensor, offset=0, ap=[[0, P], [1, C]]))
        if gates["fc2b"]:
            fc2b_sb = sg.tile([P, C], F32, tag="fc2b")
            nc.gpsimd.dma_start(out=fc2b_sb[:], in_=bass.AP(
                tensor=fc2b_d.tensor, offset=0, ap=[[0, P], [1, C]]))
        if gates["ln2b"]:
            ln2b_sb = sg.tile([P, C], F32, tag="ln2b")
            nc.gpsimd.dma_start(out=ln2b_sb[:], in_=bass.AP(
                tensor=ln2b_d.tensor, offset=0, ap=[[0, P], [1, C]]))

        eps_sb = sg.tile([P, 1], F32, tag="eps")
        nc.vector.memset(eps_sb[:], EPS)
        id32 = sg.tile([P, P], F32, tag="id32")
        make_identity(nc, id32[:])
        idr = sg.tile([P, P], F32R, tag="idr")
        nc.vector.tensor_copy(idr[:], id32[:])
        id16 = sg.tile([P, P], BF16, tag="id16")
        nc.vector.tensor_copy(id16[:], id32[:])

        # stats / mid tiles
        Sq = sg.tile([P, 2 * G_PER_B], F32, tag="Sq")
        Sk = sg.tile([P, 2 * G_PER_B], F32, tag="Sk")
        Sqq = sg.tile([P, 2 * G_PER_B], F32, tag="Sqq")
        Skk = sg.tile([P, 2 * G_PER_B], F32, tag="Skk")
        Sqk = sg.tile([P, 2 * G_PER_B], F32, tag="Sqk")
        wcols = sg.tile([P, 2 * G_PER_B], F32, tag="wcols")
        chw_sb = sg.tile([P, BL, CCH], F32, tag="chw")
        xmacc_b = [sg.tile([1, C], F32, tag=f"xmacc{b}", name=f"xmacc{b}")
                   for b in range(BL)]
        for b in range(BL):
            nc.vector.memset(xmacc_b[b][:], 0.0)
        A0pad = sg.tile([BL, N + 2], F32R, tag="A0pad")
        dma(A0pad[:, 0:1], zeros_d[0:BL, :])
        dma(A0pad[:, N + 1:N + 2], zeros_d[0:BL, :])
        h2pad = sg.tile([4 * BL, N + 2], F32R, tag="h2pad")
        dma(h2pad[:, 0:1], zeros_d)
        dma(h2pad[:, N + 1:N + 2], zeros_d)
        x1T_b = [sg.tile([P, CCH, N], F32R, tag=f"x1T{b}", name=f"x1T{b}")
                 for b in range(BL)]

        # mid scratch (one-tag singles; sliced per batch)
        MQ = sg.tile([P, 2 * G_PER_B], F32, tag="MQ")
        MK = sg.tile([P, 2 * G_PER_B], F32, tag="MK")
        T1 = sg.tile([P, 2 * G_PER_B], F32, tag="T1")
        T2 = sg.tile([P, 2 * G_PER_B], F32, tag="T2")
        T3 = sg.tile([P, 2 * G_PER_B], F32, tag="T3")
        AB = sg.tile([P, 2 * G_PER_B], F32R, tag="AB")

        def groups_of(b):
            return range(b * G_PER_B, (b + 1) * G_PER_B)

        # ================= PASS A =================
        def pass_a(b):
            for g in groups_of(b):
                gl = g - b * G_PER_B  # group within batch
                xg = wk3.tile([P, C], F32, tag="xa")
                dma(xg[:], x_d[g * P:(g + 1) * P, :])
                # LN1 stats
                bn = wk4.tile([P, 6], F32, tag="bn")
                nc.vector.bn_stats(bn[:], xg[:])
                mv = wk4.tile([P, 2], F32, tag="mv")
                nc.vector.bn_aggr(mv[:], bn[:])
                sd = wk4.tile([P, 1], F32, tag="sd")
                nc.scalar.activation(sd[:], mv[:, 1:2], AF.Sqrt, bias=eps_sb[:])
                rr = wk4.tile([P, 1], F32, tag="rr")
                nc.vector.reciprocal(rr[:], sd[:])
                x1 = wk.tile([P, C], F32R, tag="x1")
                nc.vector.tensor_scalar(x1[:], xg[:], mv[:, 0:1], rr[:],
                                        OP.subtract, OP.mult)
                # transpose x1 -> x1T_b
                for cc in range(CCH):
                    pt = ps_tr.tile([P, P], F32R, tag="tr")
                    nc.tensor.transpose(pt[:], x1[:, cc * P:(cc + 1) * P], idr[:])
                    nc.scalar.copy(x1T_b[b][:, cc, gl * P:(gl + 1) * P], pt[:])
                # xm accumulation (ones-matmul over tokens)
                pxm = ps_sm.tile([2, C], F32, tag="sm")
                nc.tensor.matmul(pxm[:], ones_sb[:], x1[:], start=True, stop=True)
                nc.vector.tensor_add(xmacc_b[b][:], xmacc_b[b][:], pxm[0:1, :])
                # q, k matmuls (orientation A: lhsT = x1T chunk, rhs = w rows)
                pq = ps_qk.tile([P, C], F32, tag="qk")
                for cc in range(CCH):
                    nc.tensor.matmul(pq[:], x1T_b[b][:, cc, gl * P:(gl + 1) * P],
                                     wqk_sb[:, cc, 0:C], start=(cc == 0),
                                     stop=(cc == CCH - 1))
                pk = ps_qk.tile([P, C], F32, tag="qk")
                for cc in range(CCH):
                    nc.tensor.matmul(pk[:], x1T_b[b][:, cc, gl * P:(gl + 1) * P],
                                     wqk_sb[:, cc, C:2 * C], start=(cc == 0),
                                     stop=(cc == CCH - 1))
                gcol = lambda S: S[:, g:g + 1]
                qsb = wk.tile([P, C], F32, tag="qsb")
                scr = wk.tile([P, C], F32, tag="scr")
                scr2 = wk.tile([P, C], F32, tag="scr2")
                if gates["qkb"]:
                    nc.vector.scalar_tensor_tensor(qsb[:], pq[:], 1.0, qb_sb[:, 0:C],
                                                   OP.mult, OP.add, accum_out=gcol(Sq))
                    ksb = wk.tile([P, C], F32, tag="ksb")
                    nc.scalar.activation(ksb[:], pk[:], AF.Identity,
                                         bias=0.0, scale=1.0)
                    nc.vector.tensor_tensor(ksb[:], ksb[:], qb_sb[:, C:2 * C], OP.add)
                    nc.vector.tensor_scalar(scr2[:], ksb[:], 1.0, 0.0, OP.mult,
                                            OP.add, accum_out=gcol(Sk))
                    nc.scalar.activation(scr[:], qsb[:], AF.Square,
                                         accum_out=gcol(Sqq))
                    nc.scalar.activation(scr2[:], ksb[:], AF.Square,
                                         accum_out=gcol(Skk))
                    nc.vector.scalar_tensor_tensor(scr[:], ksb[:], 1.0, qsb[:],
                                                   OP.mult, OP.mult,
                                                   accum_out=gcol(Sqk))
                else:
                    # q move + sum(q)
                    nc.vector.tensor_scalar(qsb[:], pq[:], 1.0, 0.0, OP.mult,
                                            OP.add, accum_out=gcol(Sq))
                    # k copy + sum(k) on ACT
                    nc.scalar.activation(scr2[:], pk[:], AF.Copy, accum_out=gcol(Sk))
                    # sum(q^2), sum(k^2) on ACT
                    nc.scalar.activation(scr[:], pq[:], AF.Square, accum_out=gcol(Sqq))
                    nc.scalar.activation(scr[:], pk[:], AF.Square, accum_out=gcol(Skk))
                    # sum(q*k) on DVE
                    nc.vector.scalar_tensor_tensor(scr2[:], pk[:], 1.0, qsb[:],
                                                   OP.mult, OP.mult,
                                                   accum_out=gcol(Sqk))

        # ================= MID =================
        def mid(b):
            bs = slice(b * G_PER_B, (b + 1) * G_PER_B)
            v = nc.vector
            # ch_w path: xm -> columns -> 2-layer MLP -> sigmoid
            xmrow = sg.tile([1, C], F32, tag=f"xmrow{b}", name=f"xmrow{b}")
            v.scalar_tensor_tensor(xmrow[:], xmacc_b[b][:], 1.0 / N, g1r_sb[:],
                                   OP.mult, OP.mult)
            if gates["ln1b"]:
                v.tensor_tensor(xmrow[:], xmrow[:], b1r_sb[:], OP.add)
            pxt = ps_sm.tile([P, CCH], F32, tag="sm")
            for cc in range(CCH):
                nc.tensor.transpose(pxt[:, cc:cc + 1],
                                    xmrow[0:1, cc * P:(cc + 1) * P], id32[0:1, 0:1])
            xmcol = sg.tile([P, CCH, 2], F32R, tag=f"xmcol{b}", name=f"xmcol{b}")
            for dup in range(2):
                v.tensor_copy(xmcol[:, :, dup], pxt[:])
            phid = ps_sm.tile([CA, 2], F32, tag="sm")
            for cc in range(CCH):
                nc.tensor.matmul(phid[:], ca1_sb[:, cc, :], xmcol[:, cc, :],
                                 start=(cc == 0), stop=(cc == CCH - 1))
            hid = sg.tile([CA, 2], F32R, tag=f"hid{b}", name=f"hid{b}")
            nc.scalar.activation(hid[:], phid[:], AF.Relu,
                                 bias=cab1_sb[:], scale=1.0)
            pchw = ps_sm.tile([P, CCH, 2], F32, tag="sm")
            for cc in range(CCH):
                nc.tensor.matmul(pchw[:, cc, :], ca2_sb[:, cc, :], hid[:],
                                 start=True, stop=True)
            if gates["cab2"]:
                for cc in range(CCH):
                    nc.scalar.activation(chw_sb[:, b, cc:cc + 1], pchw[:, cc, 0:1],
                                         AF.Sigmoid, bias=cab2_sb[:, cc:cc + 1],
                                         scale=1.0)
            else:
                nc.scalar.activation(chw_sb[:, b, :], pchw[:, :, 0], AF.Sigmoid)

            # attention stats -> attn_base (on [P, 8] col slices)
            v.tensor_scalar(MQ[:, bs], Sq[:, bs], 1.0 / C, None, OP.mult)
            v.tensor_scalar(MK[:, bs], Sk[:, bs], 1.0 / C, None, OP.mult)
            # B1 = 2*inv*(Sqk - Sq*MK) + c2
            v.tensor_mul(T1[:, bs], Sq[:, bs], MK[:, bs])
            v.tensor_sub(T1[:, bs], Sqk[:, bs], T1[:, bs])
            v.tensor_scalar(T1[:, bs], T1[:, bs], 2.0 * INV_C1, c2, OP.mult, OP.add)
            # B2 = inv*(Sqq - Sq*MQ + Skk - Sk*MK) + c2
            v.tensor_mul(T2[:, bs], Sq[:, bs], MQ[:, bs])
            v.tensor_sub(T2[:, bs], Sqq[:, bs], T2[:, bs])
            v.tensor_mul(T3[:, bs], Sk[:, bs], MK[:, bs])
            v.tensor_sub(T3[:, bs], Skk[:, bs], T3[:, bs])
            v.tensor_add(T2[:, bs], T2[:, bs], T3[:, bs])
            v.tensor_scalar(T2[:, bs], T2[:, bs], INV_C1, c2, OP.mult, OP.add)
            # A1 = 2*MQ*MK + c1 ; A2 = MQ^2 + MK^2 + c1
            v.tensor_mul(T3[:, bs], MQ[:, bs], MK[:, bs])
            v.tensor_scalar(T3[:, bs], T3[:, bs], 2.0, c1, OP.mult, OP.add)
            v.tensor_mul(T1[:, bs], T1[:, bs], T3[:, bs])   # num = A1*B1
            v.tensor_mul(T3[:, bs], MQ[:, bs], MQ[:, bs])
            v.tensor_mul(MQ[:, bs], MK[:, bs], MK[:, bs])
            v.tensor_add(T3[:, bs], T3[:, bs], MQ[:, bs])
            v.tensor_scalar(T3[:, bs], T3[:, bs], 1.0, c1, OP.mult, OP.add)
            v.tensor_mul(T2[:, bs], T2[:, bs], T3[:, bs])   # den = A2*B2
            v.tensor_scalar(T2[:, bs], T2[:, bs], 1e-7, None, OP.add)
            v.reciprocal(T2[:, bs], T2[:, bs])
            v.tensor_mul(T1[:, bs], T1[:, bs], T2[:, bs])   # ratio
            v.tensor_mul(AB[:, bs], T1[:, bs], T1[:, bs])   # attn_base (f32r)

            # transpose attn_base cols -> row layout [1, 1024] in A0pad
            pab = ps_tr.tile([G_PER_B, P], F32R, tag="tr")
            nc.tensor.transpose(pab[:], AB[:, bs], idr[:])
            ab16 = wk.tile([G_PER_B, P], F32R, tag="ab16")
            v.tensor_copy(ab16[:], pab[:])
            dma(A0pad[b:b + 1, 1:N + 1], ab16[:, :])

            # conv1 (3-tap via shifted AP slices), accumulate in PSUM halves
            ph = [ps_qk.tile([4 * BL, NT], F32, tag="qk", name=f"ph{h_}")
                  for h_ in range(2)]
            for h in range(2):
                for d in range(3):
                    nc.tensor.matmul(ph[h][:], w1d_sb[:, d, :],
                                     A0pad[:, d + h * NT:d + h * NT + NT],
                                     start=(d == 0), stop=(d == 2))
            # GroupNorm stats
            hsb = sg.tile([4 * BL, N], F32, tag="hsb")
            gscr = sg.tile([4 * BL, N], F32, tag="gt", name="gscr")
            sparts = sg.tile([4 * BL, 4], F32, tag="sparts")
            for h in range(2):
                nc.scalar.activation(hsb[:, h * NT:(h + 1) * NT], ph[h][:], AF.Copy,
                                     accum_out=sparts[:, h:h + 1])
                nc.scalar.activation(gscr[:, h * NT:(h + 1) * NT], ph[h][:], AF.Square,
                                     accum_out=sparts[:, 2 + h:3 + h])
            st = sg.tile([4 * BL, 2], F32R, tag="st")
            v.tensor_add(st[:, 0:1], sparts[:, 0:1], sparts[:, 1:2])
            v.tensor_add(st[:, 1:2], sparts[:, 2:3], sparts[:, 3:4])
            pgn = ps_sm.tile([4 * BL, 2], F32, tag="sm")
            nc.tensor.matmul(pgn[:], p8_sb[:], st[:], start=True, stop=True)
            gm = sg.tile([4 * BL, 1], F32, tag="gm")
            v.tensor_scalar(gm[:], pgn[:, 0:1], 1.0 / (2 * N), None, OP.mult)
            gv = sg.tile([4 * BL, 1], F32, tag="gv")
            v.tensor_scalar(gv[:], pgn[:, 1:2], 1.0 / (2 * N), None, OP.mult)
            gm2 = sg.tile([4 * BL, 1], F32, tag="gm2")
            v.tensor_mul(gm2[:], gm[:], gm[:])
            v.tensor_sub(gv[:], gv[:], gm2[:])
            gsd = sg.tile([4 * BL, 1], F32, tag="gsd")
            nc.scalar.activation(gsd[:], gv[:], AF.Sqrt, bias=eps_sb[0:4 * BL, :])
            gr = sg.tile([4 * BL, 1], F32, tag="gr")
            v.reciprocal(gr[:], gsd[:])
            v.tensor_mul(gr[:], gr[:], gng_sb[:])
            gt = sg.tile([4 * BL, N], F32, tag="gt")
            v.tensor_scalar(gt[:], hsb[:], gm[:], gr[:], OP.subtract, OP.mult)
            nc.scalar.activation(h2pad[:, 1:N + 1], gt[:], AF.Relu,
                                 bias=gnb_sb[:], scale=1.0)
            # conv2 + Euler + sigmoid + softmax
            pa = [ps_qk.tile([BL, NT], F32, tag="qk", name=f"pa{h_}")
                  for h_ in range(2)]
            for h in range(2):
                for d in range(3):
                    nc.tensor.matmul(pa[h][:], w2d_sb[:, d, :],
                                     h2pad[:, d + h * NT:d + h * NT + NT],
                                     start=(d == 0), stop=(d == 2))
            fa = sg.tile([BL, N], F32, tag="fa")
            for h in range(2):
                nc.vector.scalar_tensor_tensor(
                    fa[:, h * NT:(h + 1) * NT], pa[h][:], b2,
                    A0pad.bitcast(F32)[:, 1 + h * NT:1 + h * NT + NT],
                    OP.add, OP.add)
            nc.scalar.activation(fa[:], fa[:], AF.Sigmoid)
            sexp = sg.tile([BL, 1], F32, tag="sexp")
            expw = gt[0:BL, :]
            nc.scalar.activation(expw, fa[:], AF.Exp, accum_out=sexp[:])
            v.reciprocal(sexp[:], sexp[:])
            wrow = sg.tile([BL, N], F32, tag="fa", name="wrow")
            v.tensor_scalar(wrow[:], expw, sexp[:], None, OP.mult)
            # rearrange weights row b -> [8, 128] -> transpose -> wcols[:, bs]
            wg16 = wk.tile([G_PER_B, P], F32, tag="wg16")
            dma(wg16[:, :], wrow[b:b + 1, :])
            pwc = ps_tr.tile([P, G_PER_B], F32, tag="tr")
            nc.tensor.transpose(pwc[:], wg16[:], id32[0:G_PER_B, 0:G_PER_B])
            v.tensor_copy(wcols[:, bs], pwc[:])

        # ================= PASS B =================
        def pass_b(b):
            for it in range(NTILES_B):
                toff = it * NT  # token offset within batch
                # v matmuls from x1T, fused scale -> out_low (f32r)
                ol = wk.tile([P, CCH, NT], F32R, tag="ol", bufs=1)
                for mcc in range(CCH):
                    pv = ps_big.tile([P, NT], F32, tag="big")
                    for cc in range(CCH):
                        nc.tensor.matmul(pv[:], wv_sb[:, cc, mcc * P:(mcc + 1) * P],
                                         x1T_b[b][:, cc, toff:toff + NT],
                                         start=(cc == 0), stop=(cc == CCH - 1))
                    nc.vector.tensor_scalar(ol[:, mcc, :], pv[:],
                                            vb_sb[:, mcc:mcc + 1],
                                            chw_sb[:, b, mcc:mcc + 1],
                                            OP.add, OP.mult)
                x2T = wk.tile([P, CCH, NT], BF16, tag="x2T", bufs=1)
                xmids = []
                for gl in range(NT // P):
                    g = b * G_PER_B + (toff + gl * P) // P
                    grow = g * P  # global token row
                    # pl matmul -> attn_out, residual vs x (re-read)
                    ppl = ps_big.tile([P, C], F32, tag="big")
                    for cc in range(CCH):
                        nc.tensor.matmul(ppl[:], ol[:, cc, gl * P:(gl + 1) * P],
                                         pl_sb[:, cc, :], start=(cc == 0),
                                         stop=(cc == CCH - 1))
                    xb = wk3.tile([P, C], F32, tag="xa")
                    dma(xb[:], x_d[grow:grow + P, :])
                    xm_t = wk4.tile([P, C], F32, tag="xmid", bufs=4)
                    xmids.append(xm_t)
                    nc.vector.scalar_tensor_tensor(xm_t[:], ppl[:],
                                                   wcols[:, g:g + 1], xb[:],
                                                   OP.mult, OP.add)
                    if gates["plb"]:
                        nc.vector.tensor_tensor(xm_t[:], xm_t[:], plb_sb[:], OP.add)
                    # LN2
                    bn = wk4.tile([P, 6], F32, tag="bn2")
                    nc.vector.bn_stats(bn[:], xm_t[:])
                    mv = wk4.tile([P, 2], F32, tag="mv2")
                    nc.vector.bn_aggr(mv[:], bn[:])
                    sd = wk4.tile([P, 1], F32, tag="sd2")
                    nc.scalar.activation(sd[:], mv[:, 1:2], AF.Sqrt, bias=eps_sb[:])
                    rr = wk4.tile([P, 1], F32, tag="rr2")
                    nc.vector.reciprocal(rr[:], sd[:])
                    x2 = wk.tile([P, C], BF16, tag="x2")
                    if gates["ln2b"]:
                        x2f = wk.tile([P, C], F32, tag="x2f")
                        nc.vector.tensor_scalar(x2f[:], xm_t[:], mv[:, 0:1], rr[:],
                                                OP.subtract, OP.mult)
                        nc.vector.tensor_tensor(x2[:], x2f[:], ln2b_sb[:], OP.add)
                    else:
                        nc.vector.tensor_scalar(x2[:], xm_t[:], mv[:, 0:1], rr[:],
                                                OP.subtract, OP.mult)
                    # transpose x2 -> x2T (bf16)
                    for cc in range(CCH):
                        pt = ps_tr.tile([P, P], BF16, tag="tr")
                        nc.tensor.transpose(pt[:], x2[:, cc * P:(cc + 1) * P], id16[:])
                        nc.scalar.copy(x2T[:, cc, gl * P:(gl + 1) * P], pt[:])
                # fc1 + gelu -> h (bf16, feature-major, whole tile N=512)
                hsb = wk.tile([P, HCH, NT], BF16, tag="h", bufs=1)
                for hc in range(HCH):
                    pf = ps_big.tile([P, NT], F32, tag="big")
                    for cc in range(CCH):
                        nc.tensor.matmul(pf[:],
                                         fc1_sb[:, cc, hc * P:(hc + 1) * P],
                                         x2T[:, cc, :], start=(cc == 0),
                                         stop=(cc == CCH - 1))
                    nc.scalar.activation(hsb[:, hc, :], pf[:], AF.Gelu,
                                         bias=fc1b_sb[:, hc:hc + 1], scale=1.0)
                # fc2 -> + residual -> out (per group)
                for gl in range(NT // P):
                    g = b * G_PER_B + (toff + gl * P) // P
                    grow = g * P
                    po = ps_big.tile([P, C], F32, tag="big")
                    for hc in range(HCH):
                        nc.tensor.matmul(po[:], hsb[:, hc, gl * P:(gl + 1) * P],
                                         fc2_sb[:, hc, :],
                                         start=(hc == 0), stop=(hc == HCH - 1))
                    og = wk3.tile([P, C], F32, tag="og")
                    nc.vector.tensor_add(og[:], po[:], xmids[gl][:])
                    if gates["fc2b"]:
                        nc.vector.tensor_tensor(og[:], og[:], fc2b_sb[:], OP.add)
                    dma(out_d[grow:grow + P, :], og[:])

        pass_a(0)
        pass_a(1)
        mid(0)
        pass_b(0)
        mid(1)
        pass_b(1)

    nc.compile()
    return nc


_CACHE = {}


def kernel(**inputs):
    x = np.asarray(inputs["x"], np.float32)          # [16, 1024, 512]
    ln1_g = np.asarray(inputs["ln1_g"], np.float32)
    ln1_b = np.asarray(inputs["ln1_b"], np.float32)
    w_qkv = np.asarray(inputs["w_qkv"], np.float32)  # [1536, 512]
    c1 = float(np.asarray(inputs["c1"]))
    c2 = float(np.asarray(inputs["c2"]))
    ca_w1 = np.asarray(inputs["ca_w1"], np.float32)  # [128, 512]
    ca_b1 = np.asarray(inputs["ca_b1"], np.float32)
    ca_w2 = np.asarray(inputs["ca_w2"], np.float32)  # [512, 128]
    ca_b2 = np.asarray(inputs["ca_b2"], np.float32)
    ode_w1 = np.asarray(inputs["ode_w1"], np.float32)  # [4,1,3]
    gn_g = np.asarray(inputs["gn_g"], np.float32)
    gn_b = np.asarray(inputs["gn_b"], np.float32)
    ode_w2 = np.asarray(inputs["ode_w2"], np.float32)  # [1,4,3]
    ode_b2 = np.asarray(inputs["ode_b2"], np.float32)
    pl_w = np.asarray(inputs["pl_w"], np.float32)
    pl_b = np.asarray(inputs["pl_b"], np.float32)
    ln2_g = np.asarray(inputs["ln2_g"], np.float32)
    ln2_b = np.asarray(inputs["ln2_b"], np.float32)
    fc1_w = np.asarray(inputs["fc1_w"], np.float32)  # [2048, 512]
    fc1_b = np.asarray(inputs["fc1_b"], np.float32)
    fc2_w = np.asarray(inputs["fc2_w"], np.float32)  # [512, 2048]
    fc2_b = np.asarray(inputs["fc2_b"], np.float32)

    B = x.shape[0]
    ncores = 8
    bl = B // ncores

    # host-side folds
    wg = w_qkv * ln1_g[None, :]          # fold ln1_g
    qkv_b = w_qkv @ ln1_b                # [1536]
    fc1g = fc1_w * ln2_g[None, :]        # fold ln2_g
    fc1b_eff = fc1_b + fc1_w @ ln2_b

    gates = {
        "qkb": bool(np.any(qkv_b[:1024] != 0)),
        "ln1b": bool(np.any(ln1_b != 0)),
        "cab2": bool(np.any(ca_b2 != 0)),
        "plb": bool(np.any(pl_b != 0)),
        "fc2b": bool(np.any(fc2_b != 0)),
        "ln2b": bool(np.any(ln2_b != 0)),
    }
    b2v = float(ode_b2.reshape(-1)[0])

    key = (c1, c2, b2v, tuple(sorted(gates.items())))
    if key not in _CACHE:
        _CACHE[key] = _build(c1, c2, b2v, gates)
    nc = _CACHE[key]

    # conv block-diag lhsT mats
    w1d = np.zeros((bl, 3, 4 * bl), np.float32)
    w2d = np.zeros((4 * bl, 3, bl), np.float32)
    for b in range(bl):
        for co in range(4):
            for d in range(3):
                w1d[b, d, b * 4 + co] = ode_w1[co, 0, d]
        for ci in range(4):
            for d in range(3):
                w2d[b * 4 + ci, d, b] = ode_w2[0, ci, d]
    p8 = np.zeros((4 * bl, 4 * bl), np.float32)
    for p_ in range(4 * bl):
        for q_ in range(4 * bl):
            if p_ // 4 == q_ // 4 and (p_ % 4) // 2 == (q_ % 4) // 2:
                p8[p_, q_] = 1.0
    gng = np.tile(gn_g, bl).reshape(4 * bl, 1).astype(np.float32)
    gnb = np.tile(gn_b, bl).reshape(4 * bl, 1).astype(np.float32)

    common = {
        "wqkT": np.ascontiguousarray(wg[:1024].T),            # [512, 1024]
        "wvT": np.ascontiguousarray(wg[1024:].T),             # [512, 512]
        "plT": np.ascontiguousarray(pl_w.T),                  # [512, 512]
        "fc1T": np.ascontiguousarray(fc1g.T).astype(ml_dtypes.bfloat16),
        "fc2T": np.ascontiguousarray(fc2_w.T).astype(ml_dtypes.bfloat16),
        "fc1b": fc1b_eff.astype(np.float32),
        "ca1T": np.ascontiguousarray(ca_w1.T),                # [512, 128]
        "ca2T": np.ascontiguousarray(ca_w2.T),                # [128, 512]
        "cab1": ca_b1.reshape(CA, 1).astype(np.float32),
        "cab2": np.ascontiguousarray(ca_b2.reshape(CCH, P).T).astype(np.float32),
        "g1row": ln1_g.reshape(1, C).astype(np.float32),
        "b1row": ln1_b.reshape(1, C).astype(np.float32),
        "vbcol": np.ascontiguousarray(qkv_b[1024:].reshape(CCH, P).T).astype(np.float32),
        "w1d": w1d, "w2d": w2d, "p8": p8,
        "gng": gng, "gnb": gnb,
        "onesc": np.ones((P, 2), np.float32),
        "zeros8": np.zeros((4 * bl, 1), np.float32),
    }
    if gates["qkb"]:
        common["qbrow"] = qkv_b[:1024].reshape(1, 1024).astype(np.float32)
    if gates["plb"]:
        common["plbrow"] = pl_b.reshape(1, C).astype(np.float32)
    if gates["fc2b"]:
        common["fc2brow"] = fc2_b.reshape(1, C).astype(np.float32)
    if gates["ln2b"]:
        common["ln2brow"] = ln2_b.reshape(1, C).astype(np.float32)

    in_maps = []
    for i in range(ncores):
        m = dict(common)
        m["x"] = np.ascontiguousarray(
            x[i * bl:(i + 1) * bl].reshape(bl * 1024, C))
        in_maps.append(m)

    res = run_bass_kernel_spmd(nc, in_maps, core_ids=list(range(ncores)))
    out = np.stack([res.results[i]["out"] for i in range(ncores)], axis=0)
    return out.reshape(B, 1024, C).astype(np.float32)
